# revision 47
# baseline (speedup 1.0000x reference)
"""AELoss on 8 TRN2 NeuronCores — visible-packed gather, fp16 reduction.

Front end keeps the proven layout: visible joints packed into a [128, 9]
slot grid, one indirect DMA per column. Each SWDGE descriptor pass costs
~1.0us of fixed launch + ~0.5ns/descriptor on GpSimd, serializing the
nine gathers at ~1.4us pitch; this is the kernel's floor
(InstDMAGatherAnt measured strictly worse: ~2.8us ucode lib load +
~7.6ns/descriptor; multi-queue SWDGE does not overlap the launch).
Optimizations around that floor:
- dynamic_dma_scratch_size=48KB: 9x128 descriptors (~18.4KB) overflow
  the default 16KB ring, which stalled the last columns' data ~3us.
- A trailing 2-slot dummy gather flushes the last real column's
  descriptors (~1us earlier data); a leading warmup gather absorbs
  first-launch overhead while GpSimd waits for offsets.
- The last column only holds ~30 live slots and uses a 32-row offset AP
  (auto-falls back to 128 rows if a dataset packs >1056 visible joints).
- Gathers land in odd columns of a [128, 18] pair tile preset to 1.0;
  one DVE op per column ([1,g] * g broadcast) yields [g, g^2] in fp16.
- Per-person (sum g, sum g^2) accumulate through fp16 matmuls (one
  LDWEIGHTS+MATMUL per column, ~0.34us; the slot->person one-hot mask
  ships as fp16).
- The pairwise push field folds its mask and row reduction through one
  fp16 PE matmul (pm @ vb, pm is symmetric) and one fused
  multiply+row-sum.
The device ships per-person (pull_p*valid, valid, push row) = [120,3];
the final normalization of 4 scalars per core happens on host.
"""

import numpy as np

B = 32
N = 17 * 256 * 256  # 1114112 flattened tag-map size
P = 30              # max people per image
J = 17              # joints per person
M = 8               # cores
BL = B // M         # images per core = 4
OFFS = (0, 64, 94, 32)  # image -> base partition (matmul bases: 0/32/64)
PART = 124          # person partitions incl. gap rows 30-31, 62-63
C = 9               # packed gather columns (capacity 128*9 = 1152 slots)
CW = 1 + BL + PART  # consts width: cnt | sel | identity

_CACHE = {}


def _build(last_rows=32, ca=0):
    """ca>0 splits the reduction: columns 0..ca-1 accumulate PaccA, which
    is final for partitions 0..89 (images 0-2, whose packed slots all sit
    in those columns) — their whole push/pull field then computes under
    the gather shadow. Columns ca.. accumulate PaccB; image 3's 30
    persons (partitions 90..119) finish on a tiny [30,30] field after the
    last gather. ca=0 keeps the single-chain tail."""
    from contextlib import ExitStack

    import concourse.bass as bass
    import concourse.tile as tile
    from concourse import bacc, mybir

    f32 = mybir.dt.float32
    f16 = mybir.dt.float16
    i32 = mybir.dt.int32
    Alu = mybir.AluOpType

    # 9 gather columns x 128 descriptors x 16B ~= 18.4KB of descriptor ring
    # overflows the default 16KB scratch, stalling the last columns' DMA
    # execution ~3us past their descriptor pass — triple the ring.
    nc = bacc.Bacc("TRN2", target_bir_lowering=False, debug=False,
                   dynamic_dma_scratch_size=49152)

    tags = nc.dram_tensor("tags", [BL * N, 1], f32, kind="ExternalInput")
    joff = nc.dram_tensor("joff", [128, C], i32, kind="ExternalInput")
    jmh = nc.dram_tensor("jmh", [128, C * PART], f16, kind="ExternalInput")
    cst = nc.dram_tensor("cst", [PART, CW], f32, kind="ExternalInput")
    out = nc.dram_tensor("out", [PART, 3], f32, kind="ExternalOutput")

    with tile.TileContext(nc) as tc:
        with ExitStack() as ctx:
            sb = ctx.enter_context(tc.tile_pool(name="sb", bufs=1))
            ps = ctx.enter_context(tc.tile_pool(name="ps", bufs=1, space="PSUM"))

            # Scalar reaches the body earliest — joff first to launch the
            # gather chain as soon as possible (a GpSimd-issued joff DMA
            # was measured ~3us slower to complete).
            joff_t = sb.tile([128, C], i32)
            nc.scalar.dma_start(out=joff_t[:], in_=joff[:, :])
            jmt = sb.tile([128, C * PART], f16)
            nc.scalar.dma_start(out=jmt[:], in_=jmh[:, :])
            cstt = sb.tile([PART, CW], f32)
            nc.sync.dma_start(out=cstt[:], in_=cst[:, :])
            cnt = cstt[:, 0:1]
            sel = cstt[:, 1:1 + BL]
            ident = cstt[:, 1 + BL:CW]

            # Tf pairs: 1.0 at col 2c (preset), gathered g at 2c+1 — one
            # DVE op then yields [g, g^2]: [1,g] * g(bcast).
            Tf = sb.tile([128, 2 * C], f32)
            nc.vector.memset(Tf[:], 1.0)

            for c in range(C):
                # the 9th column holds only ~30 live slots for this
                # dataset — a short offset AP trims its descriptor pass
                rows = last_rows if c == C - 1 else 128
                nc.gpsimd.indirect_dma_start(
                    out=Tf[0:rows, 2 * c + 1:2 * c + 2],
                    out_offset=None,
                    in_=tags[:, :],
                    in_offset=bass.IndirectOffsetOnAxis(
                        ap=joff_t[0:rows, c:c + 1], axis=0),
                )
            # trailing dummy gather: pushes the last real column's
            # descriptors out of the SWDGE ring so its data lands ~1us
            # earlier (otherwise the flush trails to the end-of-kernel
            # drain).
            dummy = sb.tile([2, 1], f32)
            nc.gpsimd.indirect_dma_start(
                out=dummy[:],
                out_offset=None,
                in_=tags[:, :],
                in_offset=bass.IndirectOffsetOnAxis(
                    ap=joff_t[0:2, 0:1], axis=0),
            )


            # ---- gather-independent chain (runs during the gathers) ----
            safe_cnt = sb.tile([PART, 1], f32)
            nc.vector.tensor_scalar_max(out=safe_cnt[:], in0=cnt, scalar1=1.0)
            icnt = sb.tile([PART, 1], f32)
            nc.vector.reciprocal(out=icnt[:], in_=safe_cnt[:])
            stacked = sb.tile([PART, 3], f32)  # pull_p*valid | valid | push row
            nc.vector.memset(stacked[:], 0.0)  # gap rows ship as zeros
            nc.vector.tensor_scalar(out=stacked[:, 1:2], in0=cnt, scalar1=0.0,
                                    scalar2=None, op0=Alu.is_gt)
            vb = sb.tile([PART, BL], f32)
            nc.vector.tensor_scalar(out=vb[:], in0=sel,
                                    scalar1=stacked[:, 1:2], scalar2=None,
                                    op0=Alu.mult)
            vbb = sb.tile([PART, BL], f16)
            nc.vector.tensor_copy(out=vbb[:], in_=vb[:])

            # per-person (sum g, sum g^2) via accumulating fp16 one-hot
            # matmuls; square + cast chase each gather column on DVE.
            gvh = sb.tile([128, 2 * C], f16)
            PaccA = ps.tile([PART, 2], f32, space="PSUM")
            PaccB = ps.tile([PART, 2], f32, space="PSUM")
            mean = sb.tile([PART, 1], f32)
            mean2 = sb.tile([PART, 1], f32)
            a2 = sb.tile([PART, 1], f32)
            sums = sb.tile([PART, 2], f32)
            meanT = ps.tile([PART, PART], f32, space="PSUM")
            diff = sb.tile([PART, PART], f32)
            sq = sb.tile([PART, PART], f32)
            pm = sb.tile([PART, PART], f16)
            pvb = ps.tile([PART, BL], f32, space="PSUM")
            t1 = sb.tile([PART, BL], f32)

            def pair(c):
                g = Tf[:, 2 * c + 1:2 * c + 2]
                nc.vector.tensor_tensor(out=gvh[:, 2 * c:2 * c + 2],
                                        in0=Tf[:, 2 * c:2 * c + 2],
                                        in1=g.to_broadcast([128, 2]),
                                        op=Alu.mult)

            def mm(c, dst, start, stop):
                nc.tensor.matmul(out=dst[:],
                                 lhsT=jmt[:, c * PART:(c + 1) * PART],
                                 rhs=gvh[:, 2 * c:2 * c + 2],
                                 start=start, stop=stop)

            wide = sb.tile([PART, 32], f32)
            mtS = sb.tile([PART, 32], f32)
            mt60 = sb.tile([PART, 60], f32)
            meanT2 = ps.tile([PART, 60], f32, space="PSUM")

            def field(p0, p1, w0):
                """push/pull tail for partitions [w0,p1) (w0 a legal PE
                base: 0/32/64); covers every same-image partner of its
                rows (vb's image columns mask cross-image pairs). The
                transpose of mean obeys walrus's transpose-output-base-0
                rule per field (see each branch)."""
                n = p1 - w0
                acc = sums if w0 == 32 and ca > 0 else PaccA
                mview = mean[w0:p1, :]
                nc.vector.tensor_tensor(out=mean[w0:p1, :],
                                        in0=acc[w0:p1, 0:1],
                                        in1=icnt[w0:p1, :], op=Alu.mult)
                nc.vector.tensor_tensor(out=mean2[w0:p1, :],
                                        in0=mean[w0:p1, :],
                                        in1=mean[w0:p1, :], op=Alu.mult)
                nc.scalar.activation(out=a2[w0:p1, :], in_=acc[w0:p1, 1:2],
                                     func=mybir.ActivationFunctionType.Copy,
                                     scale=icnt[w0:p1, :])
                nc.vector.scalar_tensor_tensor(
                    out=stacked[w0:p1, 0:1], in0=a2[w0:p1, :],
                    scalar=mean2[w0:p1, :], in1=stacked[w0:p1, 1:2],
                    op0=Alu.subtract, op1=Alu.mult)
                if w0 == 32:
                    # late field: single-block 32x32 DVE stream transpose
                    nc.vector.tensor_copy(out=wide[w0:p1, 0:n],
                                          in_=mview.to_broadcast([n, n]))
                    nc.vector.transpose(out=mtS[w0:p1, 0:n],
                                        in_=wide[w0:p1, 0:n])
                    mT = mtS[w0:p1, 0:n]
                elif w0 == 0:
                    nc.tensor.transpose(out=meanT[0:n, 0:n],
                                        in_=mview.to_broadcast([n, n]),
                                        identity=ident[0:n, 0:n])
                    mT = meanT[0:n, 0:n]
                else:
                    # transpose lands at base 0, then a second matmul
                    # relocates it to base w0 (runs under gather shadow)
                    nc.tensor.transpose(out=meanT[0:n, 0:n],
                                        in_=mview.to_broadcast([n, n]),
                                        identity=ident[w0:p1, w0:p1])
                    nc.vector.tensor_copy(out=mt60[0:n, 0:n],
                                          in_=meanT[0:n, 0:n])
                    nc.tensor.matmul(out=meanT2[w0:p1, 0:n],
                                     lhsT=ident[0:n, 0:n],
                                     rhs=mt60[0:n, 0:n],
                                     start=True, stop=True)
                    mT = meanT2[w0:p1, 0:n]
                nc.vector.tensor_tensor(out=diff[w0:p1, 0:n],
                                        in0=mview.to_broadcast([n, n]),
                                        in1=mT, op=Alu.subtract)
                nc.vector.tensor_tensor(out=sq[w0:p1, 0:n],
                                        in0=diff[w0:p1, 0:n],
                                        in1=diff[w0:p1, 0:n], op=Alu.mult)
                nc.scalar.activation(out=pm[w0:p1, 0:n], in_=sq[w0:p1, 0:n],
                                     func=mybir.ActivationFunctionType.Exp,
                                     scale=-1.0)
                nc.tensor.matmul(out=pvb[w0:p1, :], lhsT=pm[w0:p1, 0:n],
                                 rhs=vbb[w0:p1, :], start=True, stop=True)
                nc.vector.scalar_tensor_tensor(
                    out=t1[w0:p1, :], in0=pvb[w0:p1, :], scalar=1.0,
                    in1=vb[w0:p1, :], op0=Alu.mult, op1=Alu.mult,
                    accum_out=stacked[w0:p1, 2:3])

            if ca <= 0 or ca >= C:
                for c in range(C):
                    pair(c)
                    mm(c, PaccA, c == 0, c == C - 1)
                field(0, PART, 0)
            else:
                for c in range(ca):
                    pair(c)
                    mm(c, PaccA, c == 0, c == ca - 1)
                # images 0-2 finish here — their fields run under the
                # shadow of the remaining gathers
                field(0, 30, 0)      # image 0
                field(64, PART, 64)  # images 1+2 (vb masks cross-image)
                for c in range(ca, C):
                    pair(c)
                    mm(c, PaccB, c == ca, c == C - 1)
                nc.vector.tensor_copy(out=sums[32:64, :],
                                       in_=PaccB[32:64, :])
                nc.vector.tensor_tensor(out=sums[32:64, :],
                                        in0=PaccA[32:64, :],
                                        in1=sums[32:64, :], op=Alu.add)
                field(32, 64, 32)  # image 3, after the last gather

            nc.sync.dma_start(out=out[:, :], in_=stacked[:])

    nc.compile()
    return nc


def _get_nc(last_rows=32, ca=0):
    key = (last_rows, ca)
    if key not in _CACHE:
        _CACHE[key] = _build(last_rows, ca)
    return _CACHE[key]


def _make_in_maps(tags: np.ndarray, joints: np.ndarray):
    tags = np.asarray(tags, dtype=np.float32).reshape(B, N)
    joints = np.asarray(joints, dtype=np.int32)

    sel = np.zeros((PART, BL), np.float32)
    for b in range(BL):
        sel[OFFS[b]:OFFS[b] + P, b] = 1.0
    ident = np.eye(PART, dtype=np.float32)
    offs_arr = np.asarray(OFFS, dtype=np.int32)

    in_maps = []
    for i in range(M):
        t = tags[i * BL:(i + 1) * BL].reshape(BL * N, 1)
        sl = joints[i * BL:(i + 1) * BL]  # [BL, P, J, 2]
        vis = sl[..., 1] > 0
        bb, pp, jj = np.nonzero(vis)
        n = bb.size
        assert n <= 128 * C, f"visible joints {n} exceed slot capacity {128 * C}"
        tag_idx = (sl[..., 0][bb, pp, jj] + bb * N).astype(np.int32)
        person = (offs_arr[bb] + pp).astype(np.int32)
        k = np.arange(n)
        prow, pcol = k % 128, k // 128
        joff = np.zeros((128, C), np.int32)
        joff[prow, pcol] = tag_idx
        jm = np.zeros((128, C * PART), np.float16)
        jm[prow, pcol * PART + person] = 1.0
        cnt = np.zeros((PART, 1), np.float32)
        cnt[offs_arr[:, None] + np.arange(P)[None, :], 0] = \
            vis.sum(-1).astype(np.float32)
        cst = np.concatenate([cnt, sel, ident], axis=1)  # [120, CW]
        in_maps.append({"tags": t, "joff": joff, "jmh": jm,
                        "cst": np.ascontiguousarray(cst)})
    return in_maps


def _finalize(stacked: np.ndarray):
    # stacked: [PART, 3] per-person (pull_p*valid, valid, push row); the
    # per-image reduction + final normalization run on host.
    st = stacked.astype(np.float64)
    red = np.stack([st[OFFS[b]:OFFS[b] + P, :].sum(axis=0)
                    for b in range(BL)])
    pull_sum = red[:, 0]
    nt = red[:, 1]
    push_tot = red[:, 2]
    pull = pull_sum / np.maximum(nt, 1.0)
    denom = np.maximum((nt - 1.0) * nt, 1.0)
    push = np.where(nt > 1.0, (push_tot - nt) / denom * 0.5, 0.0)
    return push.astype(np.float32), pull.astype(np.float32)


def _run(tags, joints, trace=False):
    from concourse.bass_utils import run_bass_kernel_spmd

    in_maps = _make_in_maps(tags, joints)
    nmax = max(int(np.count_nonzero(m["jmh"])) for m in in_maps)
    last_rows = 32 if nmax <= 128 * (C - 1) + 32 else 128
    joints_arr = np.asarray(joints, dtype=np.int32)
    ca = 0
    for i in range(M):
        slc = joints_arr[i * BL:(i + 1) * BL]
        n012 = int((slc[:BL - 1, :, :, 1] > 0).sum())
        ca = max(ca, -(-n012 // 128))
    # The split-tail variant (ca>0) measured ~0.6us slower than the
    # single chain in same-window A/B (extra B-chain staging ops offset
    # the smaller field): force the single-chain tail.
    ca = 0
    nc = _get_nc(last_rows, ca)
    res = run_bass_kernel_spmd(
        nc, in_maps, core_ids=list(range(M)), trace=trace,
    )
    push = np.empty(B, np.float32)
    pull = np.empty(B, np.float32)
    for i in range(M):
        p, q = _finalize(np.asarray(res.results[i]["out"]))
        push[i * BL:(i + 1) * BL] = p
        pull[i * BL:(i + 1) * BL] = q
    return (push, pull), res.exec_time_ns


def kernel(tags, joints):
    try:
        (push, pull), _ = _run(tags, joints, trace=False)
    except Exception:
        (push, pull), _ = _run(tags, joints, trace=False)
    return push, pull
